# revision 9
# baseline (speedup 1.0000x reference)
"""Row-wise cosine similarity on 8 TRN2 NeuronCores, fp16 streaming.

out[n] = sum_d(p[n,d]*h[n,d]) / (max(||p[n]||,eps) * max(||h[n]||,eps))
with N=65536, D=1024, eps=1e-12 (torch F.normalize semantics).

The f32 kernel is HBM-bound: 64 MiB/core at ~331 GB/s measured (HBM cap
per NeuronCore is ~358 GB/s) = the whole runtime. Cosine similarity
tolerates fp16 inputs easily (quantizing both vectors perturbs each cos
by ~1e-5 rms; absmax-rel error lands ~3e-4, far under the 2e-2 gate),
so the host casts to fp16 and interleaves premise/hypothesis rows into
ONE dram tensor per core laid out [128 part, T, 2, D] (row r =
partition*T + t): each group DMA is a single big contiguous-per-
partition transfer, and total traffic halves to 32 MiB/core.

At fp16 the three per-tile row-reductions (pp, hh, ph) become the
bottleneck: reduce-accumulate ops run at DVE/ACT 1x regardless of dtype
(HW-microbenched: DVE scalar_tensor_tensor+accum 1147 ns, ACT Square+
accum 1230 ns per [128,1024] tile; the DVE 2x/4x fp16 perf modes apply
only to pure tensor_tensor/copy, GPSIMD is 2-3.5 us/op, and PE cannot
reduce along the free axis). Balanced split: DVE does ph everywhere +
hh on 5/9 of tiles, ACT does pp everywhere + hh on 4/9 of tiles ->
both engines ~113 us busy over a ~101 us DMA stream.

Raw bass (no Tile scheduler); the walrus codegen accepts at most ONE
sync wait per instruction. Per group each engine's first op waits on
the group's DMA-completion sem and its last op increments that engine's
own completion sem; SP issues two consecutive wait instructions (s_act,
s_dve) before reusing a buffer slot, so the compute loops have no
cross-engine waits at all. DVE walks each group's tiles in reverse
order vs ACT's forward walk to minimise same-address SBUF read
contention. Epilogue on DVE computes res = ph * rsqrt(max(pp,eps^2) *
max(hh,eps^2)) with an ACT sqrt assist and one Newton-Raphson step,
chained op-to-op through a counting sem (every same-engine RAW is
sem-guarded; a drain-marker op observes s_dve>=NG before the final
r_ph read).
"""

import numpy as np

try:
    import concourse.bass as bass
except ImportError:  # fresh grading dir: toolchain lives in /opt
    import sys

    sys.path.insert(0, "/opt/trn_rl_repo")
    import concourse.bass as bass

from contextlib import ExitStack

from concourse import mybir
from concourse.bass_utils import run_bass_kernel_spmd

N, D = 65536, 1024
NCORES = 8
ROWS = N // NCORES  # 8192 rows per core
P = 128  # SBUF partitions
GMAX = 4  # row-tiles per full group: one [128, G, 2, D] f16 load
B = 11  # in-flight group buffers
EPS2 = 1e-24  # eps^2; max(||x||,eps) == sqrt(max(||x||^2, eps^2)) here

_NC_CACHE = {}


def _hh_on_dve(t):
    """34/64 of tiles' hh on DVE, 30/64 on ACT: balances
    64*1147+34*1147 (DVE) against 64*1188+30*1230 (ACT, pp accums in
    PSUM where the access penalty is ~42 ns/op lower). Fine-grained
    alternation — long same-engine runs let the faster engine race B
    groups ahead and stall on buffer recycling gated by the slower
    one."""
    return (t % 2 == 0) or (t % 32 == 31)


def _group_sizes(T, gmax):
    """Small first groups (compute starts early), full-size body, short
    taper so the post-stream compute drain is small."""
    front = [t for t in (2, 2) if t < gmax]
    back = [t for t in (2, 1, 1) if t < gmax]
    body = T - sum(front) - sum(back)
    assert body >= 0
    sizes = [gmax] * (body // gmax)
    rem = body % gmax
    if rem:
        sizes.append(rem)
    sizes = front + sizes + back
    assert sum(sizes) == T
    return sizes


def _build_bass(rows=ROWS, gmax=GMAX, b=B, unique_junk=False, detect_races=False):
    fp32 = mybir.dt.float32
    fp16 = mybir.dt.float16
    Sq = mybir.ActivationFunctionType.Square
    Sqrt = mybir.ActivationFunctionType.Sqrt
    mult = mybir.AluOpType.mult
    add = mybir.AluOpType.add
    T = rows // P
    B = b
    sizes = _group_sizes(T, gmax)
    starts = [sum(sizes[:i]) for i in range(len(sizes))]
    NG = len(sizes)

    nc = bass.Bass(detect_race_conditions=detect_races)
    # interleaved fp16 input: [P, T, 2, D] flattened to [P, T*2*D];
    # [:, t, 0, :] = premise row p*T+t, [:, t, 1, :] = hypothesis row
    xin = nc.declare_dram_parameter("x", [P, T * 2 * D], fp16, isOutput=False)
    outp = nc.declare_dram_parameter("out", [rows], fp32, isOutput=True)

    x4 = xin[:].rearrange("p (t c d) -> p t c d", t=T, c=2)
    out2 = outp[:].rearrange("(p t) -> p t", p=P)

    # junk: mandatory full-size outputs of accumulate ops; values unused.
    # unique_junk gives every instruction its own slice (race-detector-clean
    # validation builds only — too big for the full problem size).
    na = 2 * T if unique_junk else 1

    with ExitStack() as mem:
        xs = [
            mem.enter_context(nc.sbuf_tensor(f"xs{i}", [P, gmax, 2, D], fp16))
            for i in range(B)
        ]
        junk_a = mem.enter_context(nc.sbuf_tensor("junk_a", [P, na, D], fp16))
        junk_v = mem.enter_context(nc.sbuf_tensor("junk_v", [P, na, D], fp16))
        r_pp = mem.enter_context(nc.psum_tensor("r_pp", [P, T], fp32))
        r_hh = mem.enter_context(nc.sbuf_tensor("r_hh", [P, T], fp32))
        r_ph = mem.enter_context(nc.sbuf_tensor("r_ph", [P, T], fp32))
        r_pp2 = mem.enter_context(nc.sbuf_tensor("r_pp2", [P, T], fp32))
        d2 = mem.enter_context(nc.sbuf_tensor("d2", [P, T], fp32))
        sd = mem.enter_context(nc.sbuf_tensor("sd", [P, T], fp32))
        yv = mem.enter_context(nc.sbuf_tensor("yv", [P, T], fp32))
        t1 = mem.enter_context(nc.sbuf_tensor("t1", [P, T], fp32))
        res = mem.enter_context(nc.sbuf_tensor("res", [P, T], fp32))
        dum = mem.enter_context(nc.sbuf_tensor("dum", [P, 1], fp32))

        with ExitStack() as semctx:
            s_dma = [
                semctx.enter_context(nc.semaphore(f"s_dma{i}")) for i in range(8)
            ]
            s_act = semctx.enter_context(nc.semaphore("s_act"))
            s_dve = semctx.enter_context(nc.semaphore("s_dve"))
            s_ch = semctx.enter_context(nc.semaphore("s_ch"))
            s_ep2 = semctx.enter_context(nc.semaphore("s_ep2"))
            s_res = semctx.enter_context(nc.semaphore("s_res"))
            s_out = semctx.enter_context(nc.semaphore("s_out"))

            def xslice(g):
                s0, g0 = starts[g], sizes[g]
                return x4[:, s0 : s0 + g0, :, :]

            def dma_wait(eng, g):
                eng.wait_ge(s_dma[g % 8], 16 * (g // 8 + 1))

            with nc.Block() as block:

                @block.sync
                def _(eng: bass.BassEngine):
                    for g in range(NG):
                        if g >= B:
                            eng.wait_ge(s_act, g - B + 1)
                            eng.wait_ge(s_dve, g - B + 1)
                        eng.dma_start(
                            out=xs[g % B][:, : sizes[g], :, :], in_=xslice(g)
                        ).then_inc(s_dma[g % 8], 16)
                    eng.wait_ge(s_res, 1)
                    eng.dma_start(out=out2, in_=res[:]).then_inc(s_out, 16)
                    eng.wait_ge(s_out, 16)

                @block.scalar
                def _(eng: bass.BassEngine):
                    # pp everywhere + hh on 4/9 of tiles (ACT Square+accum)
                    for g in range(NG):
                        sl = xs[g % B]
                        dma_wait(eng, g)
                        ops = []
                        for j in range(sizes[g]):
                            t = starts[g] + j
                            ops.append((j, 0, t))  # pp
                            if not _hh_on_dve(t):
                                ops.append((j, 1, t + T))  # hh
                        for k, (j, c, tslot) in enumerate(ops):
                            ins = eng.activation(
                                out=junk_a[:, tslot % na, :],
                                in_=sl[:, j, c, :],
                                func=Sq,
                                accum_out=(r_pp if c == 0 else r_hh)[
                                    :, tslot % T : tslot % T + 1
                                ],
                            )
                            if k == len(ops) - 1:
                                ins.then_inc(s_act, 1)
                    # epilogue assist: sd = sqrt(d2)
                    eng.wait_ge(s_ch, 4)
                    eng.activation(out=sd[:], in_=d2[:], func=Sqrt).then_inc(s_ep2, 1)

                @block.vector
                def _(eng: bass.BassEngine):
                    # ph everywhere + hh on 5/9 of tiles; reverse tile order
                    # within the group to dodge ACT's forward walk
                    for g in range(NG):
                        sl = xs[g % B]
                        dma_wait(eng, g)
                        ops = []
                        for j in reversed(range(sizes[g])):
                            t = starts[g] + j
                            ops.append((j, "ph", t))
                            if _hh_on_dve(t):
                                ops.append((j, "hh", t))
                        for k, (j, kind, t) in enumerate(ops):
                            if kind == "ph":
                                ins = eng.scalar_tensor_tensor(
                                    out=junk_v[:, t % na, :],
                                    in0=sl[:, j, 0, :],
                                    scalar=1.0,
                                    in1=sl[:, j, 1, :],
                                    op0=mult,
                                    op1=mult,
                                    accum_out=r_ph[:, t : t + 1],
                                )
                            else:
                                ins = eng.scalar_tensor_tensor(
                                    out=junk_v[:, (t + T) % na, :],
                                    in0=sl[:, j, 1, :],
                                    scalar=1.0,
                                    in1=sl[:, j, 1, :],
                                    op0=mult,
                                    op1=mult,
                                    accum_out=r_hh[:, t : t + 1],
                                )
                            if k == len(ops) - 1:
                                ins.then_inc(s_dve, 1)
                    # epilogue: res = ph * rsqrt(max(pp,e)*max(hh,e)).
                    # Every same-engine RAW is sem-guarded; one wait per op.
                    # e0: drain marker — own loop (incl r_ph, r_hh) retired
                    eng.wait_ge(s_dve, NG)
                    eng.tensor_copy(out=dum[:], in_=r_ph[:, 0:1]).then_inc(s_ch, 1)
                    # e1: r_hh complete once ACT also done
                    eng.wait_ge(s_act, NG)
                    eng.tensor_scalar_max(
                        out=r_hh[:], in0=r_hh[:], scalar1=EPS2
                    ).then_inc(s_ch, 1)
                    eng.wait_ge(s_ch, 2)
                    eng.tensor_scalar_max(
                        out=r_pp2[:], in0=r_pp[:], scalar1=EPS2
                    ).then_inc(s_ch, 1)
                    eng.wait_ge(s_ch, 3)
                    eng.tensor_mul(d2[:], r_pp2[:], r_hh[:]).then_inc(s_ch, 1)
                    # (ACT: sd = sqrt(d2) after s_ch >= 4)
                    eng.wait_ge(s_ep2, 1)
                    eng.reciprocal(out=yv[:], in_=sd[:]).then_inc(s_ch, 1)
                    # Newton step for rsqrt: y *= 1.5 - 0.5*d2*y*y
                    eng.wait_ge(s_ch, 5)
                    eng.tensor_mul(t1[:], yv[:], yv[:]).then_inc(s_ch, 1)
                    eng.wait_ge(s_ch, 6)
                    eng.scalar_tensor_tensor(
                        out=t1[:], in0=d2[:], scalar=-0.5, in1=t1[:],
                        op0=mult, op1=mult,
                    ).then_inc(s_ch, 1)
                    eng.wait_ge(s_ch, 7)
                    eng.scalar_tensor_tensor(
                        out=yv[:], in0=t1[:], scalar=1.5, in1=yv[:],
                        op0=add, op1=mult,
                    ).then_inc(s_ch, 1)
                    eng.wait_ge(s_ch, 8)
                    eng.tensor_mul(res[:], r_ph[:], yv[:]).then_inc(s_res, 1)

    return nc


def _prep_inputs(premise, hypothesis, rows=ROWS):
    """Per-core fp16 interleaved [P, T*2*D] arrays."""
    T = rows // P
    cores = []
    for c in range(NCORES):
        x = np.empty((P, T, 2, D), dtype=np.float16)
        x[:, :, 0, :] = premise[c * rows : (c + 1) * rows].reshape(P, T, D)
        x[:, :, 1, :] = hypothesis[c * rows : (c + 1) * rows].reshape(P, T, D)
        cores.append({"x": x.reshape(P, T * 2 * D)})
    return cores


def _get_nc():
    if "nc" not in _NC_CACHE:
        _NC_CACHE["nc"] = _build_bass()
    return _NC_CACHE["nc"]


def _run(premise, hypothesis, trace=False, **kwargs):
    premise = np.ascontiguousarray(np.asarray(premise, dtype=np.float32))
    hypothesis = np.ascontiguousarray(np.asarray(hypothesis, dtype=np.float32))
    assert premise.shape == (N, D) and hypothesis.shape == (N, D)
    nc = _get_nc()
    in_maps = _prep_inputs(premise, hypothesis)
    r = run_bass_kernel_spmd(nc, in_maps, list(range(NCORES)), trace=trace, **kwargs)
    out = np.concatenate([r.results[c]["out"] for c in range(NCORES)])
    return out, r


def kernel(premise, hypothesis):
    out, _ = _run(premise, hypothesis)
    return out
